# Initial kernel scaffold
#
"""Trainium2 Bass kernel for nn_CotLayer (CoT attention layer, dense_cnn).

Sharding: each core computes one FULL clip (8 frames) end-to-end; cores
0-3 carry clips 0-3 and cores 4-7 are replicas (their results are
ignored).  Computing the whole clip per core makes the split-attention
GAP core-local, so there is no collective and no cross-core
synchronization -- each core's call pipeline flows independently.

All weights/constants are packed into two blob tensors (one f16, one
f32) so a call carries only 3 device arguments (x, cb16, cb32);
per-argument dispatch overhead is a large share of the end-to-end
time, not device compute.

Engine balance: ACT does the relu/silu/bias evacuations, DVE the
wd copy + the three dynamic-tap multiplies, Pool the adds and the
final attention combine; spills are batched one DMA per pixel tile.
"""
import sys
import numpy as np

try:
    import concourse.bass as bass  # noqa: F401
except ImportError:
    sys.path.insert(0, "/opt/trn_rl_repo")

import concourse.bass as bass
import concourse.tile as tile
from concourse import mybir, bacc
from concourse.bass_utils import run_bass_kernel_spmd

# ---- problem constants (hardcoded per spec) ----
C = 128          # channels
NB = 8           # temporal frames per clip
B = 4            # clips
H = W = 64
KS = 3
G = 32           # groupnorm groups = C//4
KC = 96          # KS * C//4 dynamic-kernel channels
EPS = 1e-5
NXS = NB + 2     # frames in the input shard (with zero halos)
PXF = H * W      # pixels per frame = 4096
PT = 512         # pixel tile
NT = PXF // PT   # 8 tiles per frame
NI = NT * NB     # 64 (tile, frame) iterations
NCORES = 8

F32 = mybir.dt.float32
F16 = mybir.dt.float16
AF = mybir.ActivationFunctionType
ALU = mybir.AluOpType
AXL = mybir.AxisListType

_CACHE = {}

# ---- blob column layouts (host packs / device slices must agree) ----
# cb16 (f16, [128, X16]): name -> (row_extent, col_start, col_extent)
L16 = {
    "wkey": (C, 0, KS * C),        # (i, tap, o) flattened taps
    "we1a": (C, 384, C // 2),
    "we1b": (C, 448, C // 2),
    "we2": (C, 512, KC),       # we2 stacked twice (rows 0:64 and 64:128)
    "wv": (C, 608, C),
    "bkg": (KC, 736, KS * C),      # (r, tap, c) flattened taps
}
X16 = 1120
# cb32 (f32, [128, X32])
L32 = {
    "tk": (C, 0, 1),
    "te": (C // 2, 1, 1),
    "tvc": (C, 2, NXS),
    "gavg": (KC, 12, G),
    "c1": (G, 44, C),
    "c96": (G, 172, KC),
    "be2": (KC, 268, 1),
    "ecols": (C, 269, KS * 3),
    "s2": (C, 278, 1),
    "t2": (C, 279, 1),
    "wse1": (C, 280, C),
    "b1": (C, 408, 1),
    "wsed": (C, 409, C),
    "bd": (C, 537, 1),
    "epsv": (G, 538, 1),
    "te2": (C, 539, 1),
}
X32 = 540


def _build_program(single=False, use_cc=True, reps=1):
    del use_cc  # no collective in this design
    nc = bacc.Bacc("TRN2", target_bir_lowering=False, debug=False,
                   num_devices=1 if single else NCORES)

    x_d = nc.dram_tensor("x", [C, NXS, PXF], F16, kind="ExternalInput").ap()
    cb16_d = nc.dram_tensor("cb16", [C, X16], F16, kind="ExternalInput").ap()
    cb32_d = nc.dram_tensor("cb32", [C, X32], F32, kind="ExternalInput").ap()
    out_d = nc.dram_tensor("out", [C, reps, NB, PXF], F16,
                           kind="ExternalOutput").ap()
    # spill buffers for the two branch activations between passes
    k2d_d = nc.dram_tensor("k2spill", [C, NT, NB, PT], F16)
    agg_d = nc.dram_tensor("aggspill", [C, NT, NB, PT], F16)

    with tile.TileContext(nc) as tc:
        with tc.tile_pool(name="consts", bufs=1) as cp, \
             tc.tile_pool(name="statp", bufs=2) as stp:

            cb16 = cp.tile([C, X16], F16, tag="cb16")
            nc.sync.dma_start(cb16[:], cb16_d[:])
            cb32 = cp.tile([C, X32], F32, tag="cb32")
            nc.sync.dma_start(cb32[:], cb32_d[:])

            def v16(name):
                r, c0, cn = L16[name]
                return cb16[0:r, c0:c0 + cn]

            def v32(name):
                r, c0, cn = L32[name]
                return cb32[0:r, c0:c0 + cn]

            def tap16(name, k):
                r, c0, cn = L16[name]
                w = cn // KS
                return cb16[0:r, c0 + k * w:c0 + (k + 1) * w]

            wd_big = cp.tile([KC + 1, NI, PT], F16, tag="wd_big",
                             name="wd_big")
            nc.gpsimd.memset(wd_big[KC:KC + 1, :, :], 1.0)

            for _rep in range(reps):
              stats_buf = stp.tile([KC, NT, NB, 6], F32, tag="stats",
                                   name="stats_buf")
              gap_cols = stp.tile([C, 2 * NI], F32, tag="gapc",
                                  name="gap_cols")
              s_all = [stp.tile([KC + 1, NB, C], F16, tag=f"sall{k}",
                                name=f"sall{k}")
                       for k in range(KS)]

              # ================= PASS A =================
              # frame n lives at shard slot n+1; taps use slots n..n+2
              with tc.tile_pool(name="xwA", bufs=3) as xw, \
                   tc.tile_pool(name="k2w", bufs=2) as k2w, \
                   tc.tile_pool(name="ewA", bufs=3) as ew, \
                   tc.tile_pool(name="psK", bufs=3, space="PSUM") as psK, \
                   tc.tile_pool(name="psW", bufs=3, space="PSUM") as psW, \
                   tc.tile_pool(name="psE", bufs=2, space="PSUM") as psA:
                  for t in range(NT):
                      xs_t = xw.tile([C, NXS, PT], F16, tag="xA",
                                     name="xs_t")
                      nc.sync.dma_start(
                          xs_t[:], x_d[:, :, t * PT:(t + 1) * PT])
                      k2s = k2w.tile([C, NB, PT], F16, tag="k2s")
                      # frames processed in pairs: two 64-row e tiles pack
                      # one 128-row PSUM bank -> a single relu evacuates both
                      for np_ in range(NB // 2):
                          ps_e = psA.tile([C, PT], F32, tag="ps_e")
                          et = ew.tile([C, PT], F16, tag="eA")
                          for h in range(2):
                              n = 2 * np_ + h
                              idx = t * NB + n
                              # --- key embed: temporal grouped conv ---
                              ps_k = psK.tile([C, PT], F32, tag="ps_k")
                              for k in range(KS):
                                  nc.tensor.matmul(
                                      ps_k[:], tap16("wkey", k),
                                      xs_t[:, n + k, :],
                                      start=(k == 0), stop=(k == KS - 1))
                              nc.scalar.activation(
                                  k2s[:, n, :], ps_k[:], AF.Relu,
                                  bias=v32("tk"),
                                  accum_out=gap_cols[:, idx:idx + 1])
                              # --- e = relu(bn(w_e1 @ [x; k2d])) ---
                              hr = ps_e[h * 64:(h + 1) * 64, :]
                              nc.tensor.matmul(hr, v16("we1a"),
                                               xs_t[:, n + 1, :],
                                               start=True, stop=False)
                              nc.tensor.matmul(hr, v16("we1b"),
                                               k2s[:, n, :],
                                               start=False, stop=True)
                          nc.scalar.activation(et[:], ps_e[:], AF.Relu,
                                               bias=v32("te2"))
                          for h in range(2):
                              n = 2 * np_ + h
                              idx = t * NB + n
                              # --- wd = w_e2 @ e (raw; b_e2 via stats) ---
                              ps_w = psW.tile([KC, PT], F32, tag="ps_w")
                              r0, c0, cn = L16["we2"]
                              nc.tensor.matmul(
                                  ps_w[:],
                                  cb16[h * 64:(h + 1) * 64, c0:c0 + cn],
                                  et[h * 64:(h + 1) * 64, :],
                                  start=True, stop=True)
                              if idx % 4 == 0:
                                  nc.scalar.activation(wd_big[0:KC, idx, :],
                                                       ps_w[:], AF.Copy)
                              else:
                                  nc.vector.tensor_copy(wd_big[0:KC, idx, :],
                                                        ps_w[:])
                              nc.vector.bn_stats(stats_buf[:, t, n, :],
                                                 wd_big[0:KC, idx, :])
                      nc.sync.dma_start(k2d_d.ap()[:, t, :, :], k2s[:])

              # ============ GroupNorm stats (core-local) ============
              with tc.tile_pool(name="stw", bufs=1) as sw, \
                   tc.tile_pool(name="psS", bufs=1, space="PSUM") as psS:
                  mv = sw.tile([KC, NB, 2], F32, tag="mv")
                  for n in range(NB):
                      nc.vector.bn_aggr(mv[:, n, :], stats_buf[:, :, n, :])
                  # per-channel true mean (+b_e2) and E[x^2]
                  mm96 = sw.tile([KC, 2 * NB], F32, tag="mm96")
                  nc.vector.tensor_scalar(
                      out=mm96[:, 0:NB], in0=mv[:, :, 0], scalar1=v32("be2"),
                      scalar2=None, op0=ALU.add)
                  sq = sw.tile([KC, NB], F32, tag="sq")
                  nc.vector.tensor_mul(sq[:], mm96[:, 0:NB], mm96[:, 0:NB])
                  nc.vector.tensor_add(mm96[:, NB:], mv[:, :, 1], sq[:])
                  # group stats via avg matmul
                  ps_g = psS.tile([G, 2 * NB], F32, tag="ps_g")
                  nc.tensor.matmul(ps_g[:], v32("gavg"), mm96[:],
                                   start=True, stop=True)
                  mv32 = sw.tile([G, 2 * NB], F32, tag="mv32")
                  nc.scalar.activation(mv32[:, 0:NB], ps_g[:, 0:NB],
                                       AF.Copy)
                  var = sw.tile([G, NB], F32, tag="var")
                  nc.vector.tensor_mul(var[:], mv32[:, 0:NB],
                                       mv32[:, 0:NB])
                  nc.vector.tensor_sub(var[:], ps_g[:, NB:], var[:])
                  nc.scalar.activation(var[:], var[:], AF.Sqrt,
                                       bias=v32("epsv"))
                  nc.vector.reciprocal(mv32[:, NB:], var[:])
                  # broadcast: rs to 96 rows; mu/rs to 128 channels
                  ps96 = psS.tile([KC, NB], F32, tag="ps96")
                  nc.tensor.matmul(ps96[:], v32("c96"), mv32[:, NB:],
                                   start=True, stop=True)
                  rs96 = sw.tile([KC, NB], F32, tag="rs96")
                  nc.scalar.activation(rs96[:], ps96[:], AF.Copy)
                  psc1 = psS.tile([C, 2 * NB], F32, tag="psc1")
                  nc.tensor.matmul(psc1[:], v32("c1"), mv32[:],
                                   start=True, stop=True)
                  mbrb = sw.tile([C, 2 * NB], F32, tag="mbrb")
                  nc.scalar.activation(mbrb[:], psc1[:], AF.Copy)
                  # t-bias in channel layout then scatter into S rows
                  tb = sw.tile([C, KS, NB], F16, tag="tb")
                  tba = sw.tile([C, NB], F32, tag="tba")
                  tbb = sw.tile([C, NB], F32, tag="tbb")
                  for k in range(KS):
                      ec = L32["ecols"][1]
                      nc.vector.tensor_scalar(
                          out=tba[:], in0=mbrb[:, 0:NB], scalar1=-1.0,
                          scalar2=cb32[:, ec + k * 3:ec + k * 3 + 1],
                          op0=ALU.mult, op1=ALU.add)
                      nc.vector.tensor_mul(tbb[:], tba[:], mbrb[:, NB:])
                      nc.vector.tensor_scalar(
                          out=tbb[:], in0=tbb[:],
                          scalar1=cb32[:, ec + k * 3 + 1:ec + k * 3 + 2],
                          scalar2=None, op0=ALU.mult)
                      nc.vector.tensor_scalar(
                          out=tb[:, k, :], in0=tbb[:],
                          scalar1=cb32[:, ec + k * 3 + 2:ec + k * 3 + 3],
                          scalar2=None, op0=ALU.add)
                  for k in range(KS):
                      for n in range(NB):
                          nc.vector.tensor_scalar(
                              out=s_all[k][0:KC, n, :], in0=tap16("bkg", k),
                              scalar1=rs96[:, n:n + 1], scalar2=None,
                              op0=ALU.mult)
                          nc.sync.dma_start(s_all[k][KC:KC + 1, n, :],
                                            tb[:, k, n:n + 1])

              # ================= PASS B =================
              with tc.tile_pool(name="xwB", bufs=3) as xwB, \
                   tc.tile_pool(name="vw", bufs=4) as vw, \
                   tc.tile_pool(name="mw", bufs=2) as mw, \
                   tc.tile_pool(name="agw", bufs=2) as agw, \
                   tc.tile_pool(name="psB", bufs=2, space="PSUM") as psB:
                  for t in range(NT):
                      xs_t = xwB.tile([C, NXS, PT], F16, tag="xB",
                                      name="xs_tb")
                      nc.sync.dma_start(
                          xs_t[:], x_d[:, :, t * PT:(t + 1) * PT])
                      ags = agw.tile([C, NB, PT], F16, tag="ags")
                      vt = {}

                      def makev(j, xs_t=xs_t, vt=vt):
                          ps_v = psB.tile([C, PT], F32, tag="ps_v")
                          nc.tensor.matmul(ps_v[:], v16("wv"),
                                           xs_t[:, j, :],
                                           start=True, stop=True)
                          tl = vw.tile([C, PT], F16, tag="v")
                          tvc = L32["tvc"][1]
                          nc.scalar.activation(
                              tl[:], ps_v[:], AF.Identity,
                              bias=cb32[:, tvc + j:tvc + j + 1])
                          vt[j] = tl

                      makev(1)
                      for n in range(NB):
                          idx = t * NB + n
                          if n + 2 <= NB:          # slots 0 and 9 are zero
                              makev(n + 2)
                          terms = []
                          for k in range(KS):
                              if not (1 <= n + k <= NB):
                                  continue         # tap hits a zero pad frame
                              ps_w = psB.tile([C, PT], F32, tag=f"ps_w{k}")
                              nc.tensor.matmul(
                                  ps_w[:], s_all[k][:, n, :],
                                  wd_big[:, idx, :],
                                  start=True, stop=True)
                              m = mw.tile([C, PT], F32, tag=f"m{k}")
                              nc.vector.tensor_mul(m[:], ps_w[:],
                                                   vt[n + k][:])
                              terms.append(m)
                          a2 = mw.tile([C, PT], F32, tag="accB0")
                          nc.gpsimd.tensor_add(a2[:], terms[0][:],
                                               terms[1][:])
                          if len(terms) == 3:
                              a3 = mw.tile([C, PT], F32, tag="accB1")
                              nc.gpsimd.tensor_add(a3[:], a2[:],
                                                   terms[2][:])
                          else:
                              a3 = a2
                          nc.scalar.activation(
                              ags[:, n, :], a3[:], AF.Silu, bias=v32("t2"),
                              scale=v32("s2"),
                              accum_out=gap_cols[:, NI + idx:NI + idx + 1])
                      nc.sync.dma_start(agg_d.ap()[:, t, :, :], ags[:])

              # ================= GAP + SE attention (local) ============
              with tc.tile_pool(name="sew", bufs=1) as se, \
                   tc.tile_pool(name="psE2", bufs=1, space="PSUM") as psE:
                  gap = se.tile([C, 1], F32, tag="gap")
                  nc.vector.tensor_reduce(gap[:], gap_cols[:], AXL.XYZW,
                                          ALU.add)
                  ps_a = psE.tile([C, 1], F32, tag="ps_a")
                  nc.tensor.matmul(ps_a[:], v32("wse1"), gap[:],
                                   start=True, stop=True)
                  at = se.tile([C, 1], F32, tag="at")
                  nc.scalar.activation(at[:], ps_a[:], AF.Relu,
                                       bias=v32("b1"))
                  ps_d = psE.tile([C, 1], F32, tag="ps_d")
                  nc.tensor.matmul(ps_d[:], v32("wsed"), at[:],
                                   start=True, stop=True)
                  sa = se.tile([C, 1], F32, tag="sa")
                  nc.scalar.activation(sa[:], ps_d[:], AF.Sigmoid,
                                       bias=v32("bd"))
                  sk = se.tile([C, 1], F32, tag="sk")
                  nc.vector.tensor_scalar(out=sk[:], in0=sa[:], scalar1=-1.0,
                                          scalar2=1.0, op0=ALU.mult,
                                          op1=ALU.add)

                  # ======== PASS C: out = sa*agg + sk*k2d (per channel) ====
                  with tc.tile_pool(name="cw", bufs=3) as cw, \
                       tc.tile_pool(name="mcw", bufs=2) as mcw, \
                       tc.tile_pool(name="ow", bufs=2) as ow:
                      for t in range(NT):
                          ag = cw.tile([C, NB, PT], F16, tag="agI")
                          nc.sync.dma_start(ag[:], agg_d.ap()[:, t, :, :])
                          k2 = cw.tile([C, NB, PT], F16, tag="k2I")
                          nc.sync.dma_start(k2[:], k2d_d.ap()[:, t, :, :])
                          ots = ow.tile([C, NB, PT], F16, tag="ots")
                          nc.gpsimd.tensor_scalar(
                              out=ots[:], in0=ag[:], scalar1=sa[:],
                              scalar2=None, op0=ALU.mult)
                          m2 = mcw.tile([C, NB, PT], F16, tag="mc2")
                          nc.vector.tensor_scalar(
                              out=m2[:], in0=k2[:], scalar1=sk[:],
                              scalar2=None, op0=ALU.mult)
                          nc.gpsimd.tensor_add(ots[:], ots[:], m2[:])
                          nc.sync.dma_start(
                              out_d[:, _rep, :, t * PT:(t + 1) * PT],
                              ots[:])

    nc.compile()
    return nc


def _host_constants(inp):
    f = np.float32
    d = {}
    s_k = (inp["bnk_g"] / np.sqrt(inp["bnk_v"] + EPS)).astype(f)
    t_k = (inp["bnk_b"] - inp["bnk_m"] * s_k).astype(f)
    w_key = inp["w_key"].reshape(C, C // 4, KS)          # (o, i_local, tap)
    wk = np.zeros((KS, C, C), f)
    for o in range(C):
        g = o // 32
        wk[:, 32 * g:32 * (g + 1), o] = (w_key[o].T * s_k[o])
    d["wkey"] = np.ascontiguousarray(
        wk.transpose(1, 0, 2)).reshape(C, KS * C)        # (i, tap*o)
    d["tk"] = t_k.reshape(C, 1)

    s_e = (inp["bne_g"] / np.sqrt(inp["bne_v"] + EPS)).astype(f)
    t_e = (inp["bne_b"] - inp["bne_m"] * s_e).astype(f)
    we1 = inp["w_e1"] * s_e[:, None]                      # (64, 256)
    d["we1a"] = np.ascontiguousarray(we1[:, :C].T)
    d["we1b"] = np.ascontiguousarray(we1[:, C:].T)
    d["te"] = t_e.reshape(C // 2, 1)
    d["te2"] = np.concatenate([t_e, t_e]).reshape(C, 1)
    we2t = np.ascontiguousarray(inp["w_e2"].T)           # (64, 96)
    d["we2"] = np.concatenate([we2t, we2t], axis=0)      # stacked twice

    s_1 = (inp["bn1_g"] / np.sqrt(inp["bn1_v"] + EPS)).astype(f)
    t_1 = (inp["bn1_b"] - inp["bn1_m"] * s_1).astype(f)
    d["wv"] = np.ascontiguousarray((inp["w_1x1"] * s_1[:, None]).T)
    tvc = np.zeros((C, NXS), f)
    for j in range(1, NXS - 1):
        tvc[:, j] = t_1
    d["tvc"] = tvc

    gn_g, gn_b, b_e2 = inp["gn_g"], inp["gn_b"], inp["b_e2"]
    rows = np.arange(KC)
    cols = np.arange(C)
    bkg = np.zeros((KS, KC, C), f)
    for k in range(KS):
        bkg[k] = (rows[:, None] == (3 * (cols[None, :] // 4) + k)) * \
            gn_g[rows][:, None]
    d["bkg"] = np.ascontiguousarray(
        bkg.transpose(1, 0, 2)).reshape(KC, KS * C)      # (r, tap*c)
    d["gavg"] = ((rows[:, None] // 3 == np.arange(G)[None, :]) /
                 np.float32(3.0)).astype(f)
    d["c1"] = (np.arange(G)[:, None] == (cols[None, :] // 4)).astype(f)
    d["c96"] = (np.arange(G)[:, None] == (rows[None, :] // 3)).astype(f)
    d["be2"] = b_e2.astype(f).reshape(KC, 1)
    ge = 3 * (cols // 4)
    ecols = np.zeros((C, KS, 3), f)
    for k in range(KS):
        ecols[:, k, 0] = b_e2[ge + k]
        ecols[:, k, 1] = gn_g[ge + k]
        ecols[:, k, 2] = gn_b[ge + k]
    d["ecols"] = ecols.reshape(C, KS * 3)

    s_2 = (inp["bn2_g"] / np.sqrt(inp["bn2_v"] + EPS)).astype(f)
    d["s2"] = s_2.reshape(C, 1)
    d["t2"] = (inp["bn2_b"] - inp["bn2_m"] * s_2).astype(f).reshape(C, 1)

    s_se = (inp["bnse_g"] / np.sqrt(inp["bnse_v"] + EPS)).astype(f)
    # gap in reference = mean over (N,H,W) of sum over the 2 branches; the
    # local GAP delivers the raw sum of (agg+k2d) over the clip
    wse1 = inp["w_se1"] * (s_se[:, None] / np.float32(NB * H * W))
    d["wse1"] = np.ascontiguousarray(wse1.T)
    d["b1"] = (s_se * inp["b_se1"] +
               (inp["bnse_b"] - inp["bnse_m"] * s_se)).astype(f).reshape(C, 1)
    w2 = inp["w_se2"]
    d["wsed"] = np.ascontiguousarray((w2[0::2, :] - w2[1::2, :]).T)
    d["bd"] = (inp["b_se2"][0::2] - inp["b_se2"][1::2]).astype(f).reshape(C, 1)
    d["epsv"] = np.full((G, 1), EPS, f)

    # ---- pack into the two blobs ----
    cb16 = np.zeros((C, X16), np.float16)
    for nm, (r, c0, cn) in L16.items():
        v = np.asarray(d[nm], f)
        assert v.shape == (r, cn), (nm, v.shape, (r, cn))
        cb16[0:r, c0:c0 + cn] = v.astype(np.float16)
    cb32 = np.zeros((C, X32), f)
    for nm, (r, c0, cn) in L32.items():
        v = np.asarray(d[nm], f)
        assert v.shape == (r, cn), (nm, v.shape, (r, cn))
        cb32[0:r, c0:c0 + cn] = v
    return {"cb16": cb16, "cb32": cb32}


def _shard_inputs(inputs):
    consts = _host_constants(inputs)
    x = np.asarray(inputs["x"], np.float32)
    x5 = x.reshape(B, NB, C, PXF).astype(np.float16)

    in_maps = []
    for core in range(NCORES):
        clip = core % B
        xs = np.zeros((C, NXS, PXF), np.float16)
        xs[:, 1:NXS - 1, :] = x5[clip].transpose(1, 0, 2)
        m = dict(consts)
        m["x"] = np.ascontiguousarray(xs)
        in_maps.append(m)
    return in_maps


def kernel(**inputs):
    if "nc" not in _CACHE:
        _CACHE["nc"] = _build_program()
    nc = _CACHE["nc"]

    in_maps = _shard_inputs(inputs)
    res = run_bass_kernel_spmd(nc, in_maps, list(range(NCORES)))

    out = np.empty((B, NB, C, H, W), np.float32)
    for clip in range(B):
        o = res.results[clip]["out"].reshape(C, NB, PXF)
        out[clip] = o.transpose(1, 0, 2).astype(np.float32).reshape(
            NB, C, H, W)
    return out.reshape(B * NB, C, H, W)


if __name__ == "__main__":
    sys.path.insert(0, "/root/problem")
    import reference
    inp = {k: np.asarray(v) for k, v in reference.setup_inputs().items()}
    got = kernel(**inp)
    exp = np.asarray(reference.reference(**inp))
    err = np.abs(got - exp).max() / np.abs(exp).max()
    print("abs-max relative error:", err)



# revision 1
# speedup vs baseline: 2.0627x; 2.0627x over previous
"""Trainium2 Bass kernel for nn_CotLayer (CoT attention layer, dense_cnn).

Sharding: each core computes one FULL clip (8 frames) end-to-end; cores
0-3 carry clips 0-3 and cores 4-7 are replicas (their results are
ignored).  Computing the whole clip per core makes the split-attention
GAP core-local, so there is no collective and no cross-core
synchronization -- each core's call pipeline flows independently.

All weights/constants are packed into two blob tensors (one f16, one
f32) so a call carries only 3 device arguments (x, cb16, cb32);
per-argument dispatch overhead is a large share of the end-to-end
time, not device compute.

Engine balance: ACT does the relu/silu/bias evacuations, DVE the
wd copy + the three dynamic-tap multiplies, Pool the adds and the
final attention combine; spills are batched one DMA per pixel tile.
"""
import sys
import numpy as np

try:
    import concourse.bass as bass  # noqa: F401
except ImportError:
    sys.path.insert(0, "/opt/trn_rl_repo")

import concourse.bass as bass
import concourse.tile as tile
from concourse import mybir, bacc
from concourse.bass_utils import run_bass_kernel_spmd

# ---- problem constants (hardcoded per spec) ----
C = 128          # channels
NB = 8           # temporal frames per clip
B = 4            # clips
H = W = 64
KS = 3
G = 32           # groupnorm groups = C//4
KC = 96          # KS * C//4 dynamic-kernel channels
EPS = 1e-5
NXS = NB + 2     # frames in the input shard (with zero halos)
PXF = H * W      # pixels per frame = 4096
PT = 512         # pixel tile
NT = PXF // PT   # 8 tiles per frame
NI = NT * NB     # 64 (tile, frame) iterations
NCORES = 8

F32 = mybir.dt.float32
F16 = mybir.dt.float16
AF = mybir.ActivationFunctionType
ALU = mybir.AluOpType
AXL = mybir.AxisListType

_CACHE = {}

# ---- blob column layouts (host packs / device slices must agree) ----
# cb16 (f16, [128, X16]): name -> (row_extent, col_start, col_extent)
L16 = {
    "wkey": (C, 0, KS * C),        # (i, tap, o) flattened taps
    "we1a": (C, 384, C // 2),
    "we1b": (C, 448, C // 2),
    "we2": (C, 512, KC),       # we2 stacked twice (rows 0:64 and 64:128)
    "wv": (C, 608, C),
    "bkg": (KC, 736, KS * C),      # (r, tap, c) flattened taps
}
X16 = 1120
# cb32 (f32, [128, X32])
L32 = {
    "tk": (C, 0, 1),
    "te": (C // 2, 1, 1),
    "tvc": (C, 2, NXS),
    "gavg": (KC, 12, G),
    "c1": (G, 44, C),
    "c96": (G, 172, KC),
    "be2": (KC, 268, 1),
    "ecols": (C, 269, KS * 3),
    "s2": (C, 278, 1),
    "t2": (C, 279, 1),
    "wse1": (C, 280, C),
    "b1": (C, 408, 1),
    "wsed": (C, 409, C),
    "bd": (C, 537, 1),
    "epsv": (G, 538, 1),
    "te2": (C, 539, 1),
}
X32 = 540


def _build_program(single=False, use_cc=True, reps=1):
    del use_cc  # no collective in this design
    nc = bacc.Bacc("TRN2", target_bir_lowering=False, debug=False,
                   num_devices=1 if single else NCORES)

    x_d = nc.dram_tensor("x", [C, NXS, PXF], F16, kind="ExternalInput").ap()
    cb16_d = nc.dram_tensor("cb16", [C, X16], F16, kind="ExternalInput").ap()
    cb32_d = nc.dram_tensor("cb32", [C, X32], F32, kind="ExternalInput").ap()
    out_d = nc.dram_tensor("out", [C, reps, NB, PXF], F16,
                           kind="ExternalOutput").ap()
    # spill buffers for the two branch activations between passes
    k2d_d = nc.dram_tensor("k2spill", [C, NT, NB, PT], F16)
    agg_d = nc.dram_tensor("aggspill", [C, NT, NB, PT], F16)

    with tile.TileContext(nc) as tc:
        with tc.tile_pool(name="consts", bufs=1) as cp, \
             tc.tile_pool(name="statp", bufs=2) as stp:

            cb16 = cp.tile([C, X16], F16, tag="cb16")
            nc.sync.dma_start(cb16[:], cb16_d[:])
            cb32 = cp.tile([C, X32], F32, tag="cb32")
            nc.sync.dma_start(cb32[:], cb32_d[:])

            def v16(name):
                r, c0, cn = L16[name]
                return cb16[0:r, c0:c0 + cn]

            def v32(name):
                r, c0, cn = L32[name]
                return cb32[0:r, c0:c0 + cn]

            def tap16(name, k):
                r, c0, cn = L16[name]
                w = cn // KS
                return cb16[0:r, c0 + k * w:c0 + (k + 1) * w]

            wd_big = cp.tile([KC + 1, NI, PT], F16, tag="wd_big",
                             name="wd_big")
            nc.gpsimd.memset(wd_big[KC:KC + 1, :, :], 1.0)

            for _rep in range(reps):
              stats_buf = stp.tile([KC, NT, NB, 6], F32, tag="stats",
                                   name="stats_buf")
              gap_cols = stp.tile([C, 2 * NI], F32, tag="gapc",
                                  name="gap_cols")
              s_all = [stp.tile([KC + 1, NB, C], F16, tag=f"sall{k}",
                                name=f"sall{k}")
                       for k in range(KS)]

              # ================= PASS A =================
              # frame n lives at shard slot n+1; taps use slots n..n+2
              with tc.tile_pool(name="xwA", bufs=3) as xw, \
                   tc.tile_pool(name="k2w", bufs=2) as k2w, \
                   tc.tile_pool(name="ewA", bufs=3) as ew, \
                   tc.tile_pool(name="psK", bufs=3, space="PSUM") as psK, \
                   tc.tile_pool(name="psW", bufs=3, space="PSUM") as psW, \
                   tc.tile_pool(name="psE", bufs=2, space="PSUM") as psA:
                  for t in range(NT):
                      xs_t = xw.tile([C, NXS, PT], F16, tag="xA",
                                     name="xs_t")
                      nc.sync.dma_start(
                          xs_t[:], x_d[:, :, t * PT:(t + 1) * PT])
                      k2s = k2w.tile([C, NB, PT], F16, tag="k2s")
                      # frames processed in pairs: two 64-row e tiles pack
                      # one 128-row PSUM bank -> a single relu evacuates both
                      for np_ in range(NB // 2):
                          ps_e = psA.tile([C, PT], F32, tag="ps_e")
                          et = ew.tile([C, PT], F16, tag="eA")
                          for h in range(2):
                              n = 2 * np_ + h
                              idx = t * NB + n
                              # --- key embed: temporal grouped conv ---
                              ps_k = psK.tile([C, PT], F32, tag="ps_k")
                              for k in range(KS):
                                  nc.tensor.matmul(
                                      ps_k[:], tap16("wkey", k),
                                      xs_t[:, n + k, :],
                                      start=(k == 0), stop=(k == KS - 1))
                              nc.scalar.activation(
                                  k2s[:, n, :], ps_k[:], AF.Relu,
                                  bias=v32("tk"),
                                  accum_out=gap_cols[:, idx:idx + 1])
                              # --- e = relu(bn(w_e1 @ [x; k2d])) ---
                              hr = ps_e[h * 64:(h + 1) * 64, :]
                              nc.tensor.matmul(hr, v16("we1a"),
                                               xs_t[:, n + 1, :],
                                               start=True, stop=False)
                              nc.tensor.matmul(hr, v16("we1b"),
                                               k2s[:, n, :],
                                               start=False, stop=True)
                          nc.scalar.activation(et[:], ps_e[:], AF.Relu,
                                               bias=v32("te2"))
                          for h in range(2):
                              n = 2 * np_ + h
                              idx = t * NB + n
                              # --- wd = w_e2 @ e (raw; b_e2 via stats) ---
                              ps_w = psW.tile([KC, PT], F32, tag="ps_w")
                              r0, c0, cn = L16["we2"]
                              nc.tensor.matmul(
                                  ps_w[:],
                                  cb16[h * 64:(h + 1) * 64, c0:c0 + cn],
                                  et[h * 64:(h + 1) * 64, :],
                                  start=True, stop=True)
                              if idx % 4 == 0:
                                  nc.scalar.activation(wd_big[0:KC, idx, :],
                                                       ps_w[:], AF.Copy)
                              else:
                                  nc.vector.tensor_copy(wd_big[0:KC, idx, :],
                                                        ps_w[:])
                              nc.vector.bn_stats(stats_buf[:, t, n, :],
                                                 wd_big[0:KC, idx, :])
                      nc.sync.dma_start(k2d_d.ap()[:, t, :, :], k2s[:])

              # ============ GroupNorm stats (core-local) ============
              with tc.tile_pool(name="stw", bufs=1) as sw, \
                   tc.tile_pool(name="psS", bufs=1, space="PSUM") as psS:
                  mv = sw.tile([KC, NB, 2], F32, tag="mv")
                  for n in range(NB):
                      nc.vector.bn_aggr(mv[:, n, :], stats_buf[:, :, n, :])
                  # per-channel true mean (+b_e2) and E[x^2]
                  mm96 = sw.tile([KC, 2 * NB], F32, tag="mm96")
                  nc.vector.tensor_scalar(
                      out=mm96[:, 0:NB], in0=mv[:, :, 0], scalar1=v32("be2"),
                      scalar2=None, op0=ALU.add)
                  sq = sw.tile([KC, NB], F32, tag="sq")
                  nc.vector.tensor_mul(sq[:], mm96[:, 0:NB], mm96[:, 0:NB])
                  nc.vector.tensor_add(mm96[:, NB:], mv[:, :, 1], sq[:])
                  # group stats via avg matmul
                  ps_g = psS.tile([G, 2 * NB], F32, tag="ps_g")
                  nc.tensor.matmul(ps_g[:], v32("gavg"), mm96[:],
                                   start=True, stop=True)
                  mv32 = sw.tile([G, 2 * NB], F32, tag="mv32")
                  nc.scalar.activation(mv32[:, 0:NB], ps_g[:, 0:NB],
                                       AF.Copy)
                  var = sw.tile([G, NB], F32, tag="var")
                  nc.vector.tensor_mul(var[:], mv32[:, 0:NB],
                                       mv32[:, 0:NB])
                  nc.vector.tensor_sub(var[:], ps_g[:, NB:], var[:])
                  nc.scalar.activation(var[:], var[:], AF.Sqrt,
                                       bias=v32("epsv"))
                  nc.vector.reciprocal(mv32[:, NB:], var[:])
                  # broadcast: rs to 96 rows; mu/rs to 128 channels
                  ps96 = psS.tile([KC, NB], F32, tag="ps96")
                  nc.tensor.matmul(ps96[:], v32("c96"), mv32[:, NB:],
                                   start=True, stop=True)
                  rs96 = sw.tile([KC, NB], F32, tag="rs96")
                  nc.scalar.activation(rs96[:], ps96[:], AF.Copy)
                  psc1 = psS.tile([C, 2 * NB], F32, tag="psc1")
                  nc.tensor.matmul(psc1[:], v32("c1"), mv32[:],
                                   start=True, stop=True)
                  mbrb = sw.tile([C, 2 * NB], F32, tag="mbrb")
                  nc.scalar.activation(mbrb[:], psc1[:], AF.Copy)
                  # t-bias in channel layout then scatter into S rows
                  tb = sw.tile([C, KS, NB], F16, tag="tb")
                  tba = sw.tile([C, NB], F32, tag="tba")
                  tbb = sw.tile([C, NB], F32, tag="tbb")
                  for k in range(KS):
                      ec = L32["ecols"][1]
                      nc.vector.tensor_scalar(
                          out=tba[:], in0=mbrb[:, 0:NB], scalar1=-1.0,
                          scalar2=cb32[:, ec + k * 3:ec + k * 3 + 1],
                          op0=ALU.mult, op1=ALU.add)
                      nc.vector.tensor_mul(tbb[:], tba[:], mbrb[:, NB:])
                      nc.vector.tensor_scalar(
                          out=tbb[:], in0=tbb[:],
                          scalar1=cb32[:, ec + k * 3 + 1:ec + k * 3 + 2],
                          scalar2=None, op0=ALU.mult)
                      nc.vector.tensor_scalar(
                          out=tb[:, k, :], in0=tbb[:],
                          scalar1=cb32[:, ec + k * 3 + 2:ec + k * 3 + 3],
                          scalar2=None, op0=ALU.add)
                  for k in range(KS):
                      for n in range(NB):
                          nc.vector.tensor_scalar(
                              out=s_all[k][0:KC, n, :], in0=tap16("bkg", k),
                              scalar1=rs96[:, n:n + 1], scalar2=None,
                              op0=ALU.mult)
                          nc.sync.dma_start(s_all[k][KC:KC + 1, n, :],
                                            tb[:, k, n:n + 1])

              # ================= PASS B =================
              with tc.tile_pool(name="xwB", bufs=3) as xwB, \
                   tc.tile_pool(name="vw", bufs=4) as vw, \
                   tc.tile_pool(name="mw", bufs=2) as mw, \
                   tc.tile_pool(name="agw", bufs=2) as agw, \
                   tc.tile_pool(name="psB", bufs=2, space="PSUM") as psB:
                  for t in range(NT):
                      xs_t = xwB.tile([C, NXS, PT], F16, tag="xB",
                                      name="xs_tb")
                      nc.sync.dma_start(
                          xs_t[:], x_d[:, :, t * PT:(t + 1) * PT])
                      ags = agw.tile([C, NB, PT], F16, tag="ags")
                      vt = {}

                      def makev(j, xs_t=xs_t, vt=vt):
                          ps_v = psB.tile([C, PT], F32, tag="ps_v")
                          nc.tensor.matmul(ps_v[:], v16("wv"),
                                           xs_t[:, j, :],
                                           start=True, stop=True)
                          tl = vw.tile([C, PT], F16, tag="v")
                          tvc = L32["tvc"][1]
                          nc.scalar.activation(
                              tl[:], ps_v[:], AF.Identity,
                              bias=cb32[:, tvc + j:tvc + j + 1])
                          vt[j] = tl

                      makev(1)
                      for n in range(NB):
                          idx = t * NB + n
                          if n + 2 <= NB:          # slots 0 and 9 are zero
                              makev(n + 2)
                          terms = []
                          for k in range(KS):
                              if not (1 <= n + k <= NB):
                                  continue         # tap hits a zero pad frame
                              ps_w = psB.tile([C, PT], F32, tag=f"ps_w{k}")
                              nc.tensor.matmul(
                                  ps_w[:], s_all[k][:, n, :],
                                  wd_big[:, idx, :],
                                  start=True, stop=True)
                              m = mw.tile([C, PT], F32, tag=f"m{k}")
                              nc.vector.tensor_mul(m[:], ps_w[:],
                                                   vt[n + k][:])
                              terms.append(m)
                          a2 = mw.tile([C, PT], F32, tag="accB0")
                          nc.gpsimd.tensor_add(a2[:], terms[0][:],
                                               terms[1][:])
                          if len(terms) == 3:
                              a3 = mw.tile([C, PT], F32, tag="accB1")
                              nc.gpsimd.tensor_add(a3[:], a2[:],
                                                   terms[2][:])
                          else:
                              a3 = a2
                          nc.scalar.activation(
                              ags[:, n, :], a3[:], AF.Silu, bias=v32("t2"),
                              scale=v32("s2"),
                              accum_out=gap_cols[:, NI + idx:NI + idx + 1])
                      nc.sync.dma_start(agg_d.ap()[:, t, :, :], ags[:])

              # ================= GAP + SE attention (local) ============
              with tc.tile_pool(name="sew", bufs=1) as se, \
                   tc.tile_pool(name="psE2", bufs=1, space="PSUM") as psE:
                  gap = se.tile([C, 1], F32, tag="gap")
                  nc.vector.tensor_reduce(gap[:], gap_cols[:], AXL.XYZW,
                                          ALU.add)
                  ps_a = psE.tile([C, 1], F32, tag="ps_a")
                  nc.tensor.matmul(ps_a[:], v32("wse1"), gap[:],
                                   start=True, stop=True)
                  at = se.tile([C, 1], F32, tag="at")
                  nc.scalar.activation(at[:], ps_a[:], AF.Relu,
                                       bias=v32("b1"))
                  ps_d = psE.tile([C, 1], F32, tag="ps_d")
                  nc.tensor.matmul(ps_d[:], v32("wsed"), at[:],
                                   start=True, stop=True)
                  sa = se.tile([C, 1], F32, tag="sa")
                  nc.scalar.activation(sa[:], ps_d[:], AF.Sigmoid,
                                       bias=v32("bd"))
                  sk = se.tile([C, 1], F32, tag="sk")
                  nc.vector.tensor_scalar(out=sk[:], in0=sa[:], scalar1=-1.0,
                                          scalar2=1.0, op0=ALU.mult,
                                          op1=ALU.add)

                  # ======== PASS C: out = sa*agg + sk*k2d (per channel) ====
                  with tc.tile_pool(name="cw", bufs=3) as cw, \
                       tc.tile_pool(name="mcw", bufs=2) as mcw, \
                       tc.tile_pool(name="ow", bufs=2) as ow:
                      for t in range(NT):
                          ag = cw.tile([C, NB, PT], F16, tag="agI")
                          nc.sync.dma_start(ag[:], agg_d.ap()[:, t, :, :])
                          k2 = cw.tile([C, NB, PT], F16, tag="k2I")
                          nc.sync.dma_start(k2[:], k2d_d.ap()[:, t, :, :])
                          ots = ow.tile([C, NB, PT], F16, tag="ots")
                          nc.gpsimd.tensor_scalar(
                              out=ots[:], in0=ag[:], scalar1=sa[:],
                              scalar2=None, op0=ALU.mult)
                          m2 = mcw.tile([C, NB, PT], F16, tag="mc2")
                          nc.vector.tensor_scalar(
                              out=m2[:], in0=k2[:], scalar1=sk[:],
                              scalar2=None, op0=ALU.mult)
                          nc.gpsimd.tensor_add(ots[:], ots[:], m2[:])
                          nc.sync.dma_start(
                              out_d[:, _rep, :, t * PT:(t + 1) * PT],
                              ots[:])

    nc.compile()
    return nc


def _host_constants(inp):
    f = np.float32
    d = {}
    s_k = (inp["bnk_g"] / np.sqrt(inp["bnk_v"] + EPS)).astype(f)
    t_k = (inp["bnk_b"] - inp["bnk_m"] * s_k).astype(f)
    w_key = inp["w_key"].reshape(C, C // 4, KS)          # (o, i_local, tap)
    wk = np.zeros((KS, C, C), f)
    for o in range(C):
        g = o // 32
        wk[:, 32 * g:32 * (g + 1), o] = (w_key[o].T * s_k[o])
    d["wkey"] = np.ascontiguousarray(
        wk.transpose(1, 0, 2)).reshape(C, KS * C)        # (i, tap*o)
    d["tk"] = t_k.reshape(C, 1)

    s_e = (inp["bne_g"] / np.sqrt(inp["bne_v"] + EPS)).astype(f)
    t_e = (inp["bne_b"] - inp["bne_m"] * s_e).astype(f)
    we1 = inp["w_e1"] * s_e[:, None]                      # (64, 256)
    d["we1a"] = np.ascontiguousarray(we1[:, :C].T)
    d["we1b"] = np.ascontiguousarray(we1[:, C:].T)
    d["te"] = t_e.reshape(C // 2, 1)
    d["te2"] = np.concatenate([t_e, t_e]).reshape(C, 1)
    we2t = np.ascontiguousarray(inp["w_e2"].T)           # (64, 96)
    d["we2"] = np.concatenate([we2t, we2t], axis=0)      # stacked twice

    s_1 = (inp["bn1_g"] / np.sqrt(inp["bn1_v"] + EPS)).astype(f)
    t_1 = (inp["bn1_b"] - inp["bn1_m"] * s_1).astype(f)
    d["wv"] = np.ascontiguousarray((inp["w_1x1"] * s_1[:, None]).T)
    tvc = np.zeros((C, NXS), f)
    for j in range(1, NXS - 1):
        tvc[:, j] = t_1
    d["tvc"] = tvc

    gn_g, gn_b, b_e2 = inp["gn_g"], inp["gn_b"], inp["b_e2"]
    rows = np.arange(KC)
    cols = np.arange(C)
    bkg = np.zeros((KS, KC, C), f)
    for k in range(KS):
        bkg[k] = (rows[:, None] == (3 * (cols[None, :] // 4) + k)) * \
            gn_g[rows][:, None]
    d["bkg"] = np.ascontiguousarray(
        bkg.transpose(1, 0, 2)).reshape(KC, KS * C)      # (r, tap*c)
    d["gavg"] = ((rows[:, None] // 3 == np.arange(G)[None, :]) /
                 np.float32(3.0)).astype(f)
    d["c1"] = (np.arange(G)[:, None] == (cols[None, :] // 4)).astype(f)
    d["c96"] = (np.arange(G)[:, None] == (rows[None, :] // 3)).astype(f)
    d["be2"] = b_e2.astype(f).reshape(KC, 1)
    ge = 3 * (cols // 4)
    ecols = np.zeros((C, KS, 3), f)
    for k in range(KS):
        ecols[:, k, 0] = b_e2[ge + k]
        ecols[:, k, 1] = gn_g[ge + k]
        ecols[:, k, 2] = gn_b[ge + k]
    d["ecols"] = ecols.reshape(C, KS * 3)

    s_2 = (inp["bn2_g"] / np.sqrt(inp["bn2_v"] + EPS)).astype(f)
    d["s2"] = s_2.reshape(C, 1)
    d["t2"] = (inp["bn2_b"] - inp["bn2_m"] * s_2).astype(f).reshape(C, 1)

    s_se = (inp["bnse_g"] / np.sqrt(inp["bnse_v"] + EPS)).astype(f)
    # gap in reference = mean over (N,H,W) of sum over the 2 branches; the
    # local GAP delivers the raw sum of (agg+k2d) over the clip
    wse1 = inp["w_se1"] * (s_se[:, None] / np.float32(NB * H * W))
    d["wse1"] = np.ascontiguousarray(wse1.T)
    d["b1"] = (s_se * inp["b_se1"] +
               (inp["bnse_b"] - inp["bnse_m"] * s_se)).astype(f).reshape(C, 1)
    w2 = inp["w_se2"]
    d["wsed"] = np.ascontiguousarray((w2[0::2, :] - w2[1::2, :]).T)
    d["bd"] = (inp["b_se2"][0::2] - inp["b_se2"][1::2]).astype(f).reshape(C, 1)
    d["epsv"] = np.full((G, 1), EPS, f)

    # ---- pack into the two blobs ----
    cb16 = np.zeros((C, X16), np.float16)
    for nm, (r, c0, cn) in L16.items():
        v = np.asarray(d[nm], f)
        assert v.shape == (r, cn), (nm, v.shape, (r, cn))
        cb16[0:r, c0:c0 + cn] = v.astype(np.float16)
    cb32 = np.zeros((C, X32), f)
    for nm, (r, c0, cn) in L32.items():
        v = np.asarray(d[nm], f)
        assert v.shape == (r, cn), (nm, v.shape, (r, cn))
        cb32[0:r, c0:c0 + cn] = v
    return {"cb16": cb16, "cb32": cb32}


def _shard_inputs(inputs):
    consts = _host_constants(inputs)
    x = np.asarray(inputs["x"], np.float32)
    x5 = x.reshape(B, NB, C, PXF).astype(np.float16)

    in_maps = []
    for core in range(NCORES):
        clip = core % B
        xs = np.zeros((C, NXS, PXF), np.float16)
        xs[:, 1:NXS - 1, :] = x5[clip].transpose(1, 0, 2)
        m = dict(consts)
        m["x"] = np.ascontiguousarray(xs)
        in_maps.append(m)
    return in_maps


def kernel(**inputs):
    if "nc" not in _CACHE:
        _CACHE["nc"] = _build_program()
    nc = _CACHE["nc"]

    in_maps = _shard_inputs(inputs)
    res = run_bass_kernel_spmd(nc, in_maps, list(range(NCORES)))

    out = np.empty((B, NB, C, H, W), np.float32)
    for clip in range(B):
        o = res.results[clip]["out"].reshape(C, NB, PXF)
        out[clip] = o.transpose(1, 0, 2).astype(np.float32).reshape(
            NB, C, H, W)
    return out.reshape(B * NB, C, H, W)


if __name__ == "__main__":
    sys.path.insert(0, "/root/problem")
    import reference
    inp = {k: np.asarray(v) for k, v in reference.setup_inputs().items()}
    got = kernel(**inp)
    exp = np.asarray(reference.reference(**inp))
    err = np.abs(got - exp).max() / np.abs(exp).max()
    print("abs-max relative error:", err)

